# revision 9
# baseline (speedup 1.0000x reference)
"""CFAR box-filter kernel (31x31 / 11x11 box sums + ratio) for Trainium2.

Data-parallel over batch: 32 images -> 8 NeuronCores, 4 images each.
v3 design:
  - horizontal windowed sums via ONE custom DVE op per window size
    (BOXDIFF: out = cumsum(in0) - cumsum(in1), in0/in1 the same
    zero-padded row shifted by the window length),
  - bf16 datapath end-to-end: bf16 HBM input, bf16 h tiles, bf16
    outputs (host upcasts to fp32),
  - vertical box sums as banded bf16 matmuls; halo terms go through a
    single zero-padded [128,W] gather tile so every matmul keeps a
    128-row moving operand (full PE clock; small moving operands
    drop the PE out of its high p-state),
  - weight scales: wf = bf16(1/121) compensated exactly by the ACT
    copy scale; wb = +-bf16(1/840) leaves a 2.4e-4 constant factor on
    the ratio (far under tolerance),
  - flat chunk loop across images, produce lookahead 3, gathers issued
    from the GpSimd DGE one iteration ahead of their consumer.
"""

import os
import sys

import numpy as np

for _p in ("/opt/trn_rl_repo", "/root/.axon_site/_ro/trn_rl_repo"):
    if os.path.isdir(_p) and _p not in sys.path:
        sys.path.insert(0, _p)
        break

import ml_dtypes

import concourse.bass as bass
import concourse.tile as tile
from concourse import bacc, dve_ops, mybir
from concourse._compat import with_exitstack
from concourse.bass_utils import run_bass_kernel_spmd
from concourse.dve_spec import AluOp, Spec, Src0, Src1, lower, scan

B, H, W = 32, 1024, 1024
NCORES = 8
BPC = B // NCORES            # images per core
CHUNKS = H // 128            # row chunks per image
TOT = BPC * CHUNKS           # chunks per core
PADL, PADR = 31, 15
SCANW = PADL + W + PADR      # 1070
XW = 2 * SCANW               # two chunks per x-load DMA
N31 = 15 + W                 # h31 tile width (first 15 cols are scan warmup)
N11 = 25 + W                 # h11 tile width (first 25 cols are scan warmup)
O31, O11 = 15, 25            # valid-column offsets into h31/h11

F32 = mybir.dt.float32
BF16 = mybir.dt.bfloat16
BF = ml_dtypes.bfloat16

R_IN, R_OUT = 5, 15
S_F = float(BF(1.0 / 121.0))          # front weight scale (bf16 value)
S_B = float(BF(1.0 / 840.0))          # back weight scale (bf16 value)
C_FRONT = 1.0 / (121.0 * S_F)         # exact fp32 fix-up on the ACT copy
# ratio = psf*recip(psb) = front/back * 1/(840*S_B); 840*S_B = 0.99976...


def _register_boxdiff():
    for op in dve_ops.OPS:
        if op.name == "BOXDIFF_ANT":
            return op
    opcode = dve_ops._CUSTOM_DVE_ROW_BASE + len(dve_ops.OPS)
    spec = Spec(
        body=scan(AluOp.ADD, Src0) - scan(AluOp.ADD, Src1),
        reference=lambda in0, in1, s0, s1, imm2: (
            np.cumsum(in0, -1) - np.cumsum(in1, -1)
        ),
    )
    from concourse.dve_uop import DveOpSpec

    shas = {}
    for ver in ("v3", "v4"):
        s = DveOpSpec(
            name="BOXDIFF_ANT", opcode=opcode, uops=lower(spec, ver=ver), rd1_en=True
        )
        shas[ver] = s.sha(ver)
    op = dve_ops.DveOp("BOXDIFF_ANT", spec, subdim=False, uops_sha=shas)
    dve_ops.OPS.append(op)
    dve_ops.CUSTOM_DVE_SPECS[op.name] = spec
    dve_ops._SUB_OPCODE_FOR_NAME[op.name] = opcode
    return op


BOXDIFF = _register_boxdiff()

# gather-tile row layout (fixed): h31 halo at [0:30], h11 halo at [64:74]
G31P, G31N = slice(0, 15), slice(15, 30)
G11P, G11N = slice(64, 69), slice(69, 74)


def _weights() -> dict[str, np.ndarray]:
    k = np.arange(128)[:, None]
    m = np.arange(128)[None, :]
    g11 = np.arange(2 * R_IN)[:, None]
    g11 = np.where(g11 < R_IN, g11 - R_IN, 128 + (g11 - R_IN))
    g31 = np.arange(2 * R_OUT)[:, None]
    g31 = np.where(g31 < R_OUT, g31 - R_OUT, 128 + (g31 - R_OUT))

    def band(gg, radius, scale):
        return ((np.abs(gg - m) <= radius) * scale).astype(BF)

    wb31_h = band(g31, R_OUT, S_B)       # [30,128] rows: prev15, next15
    wn11_h = band(g11, R_IN, -S_B)       # [10,128] rows: prev5, next5
    wf_h = band(g11, R_IN, S_F)          # [10,128]

    def mk_bh(pn):
        wb = np.zeros((128, 128), dtype=BF)
        if pn != "N":
            wb[0:15] = wb31_h[:15]
            wb[G11P] = wn11_h[:5]
        if pn != "P":
            wb[15:30] = wb31_h[15:]
            wb[G11N] = wn11_h[5:]
        return wb

    def mk_fh(pn):
        # stored as [128,128]; the matmul uses rows [64:128] so the weight
        # tile's base partition matches the gather tile's h11-halo rows
        wf = np.zeros((128, 128), dtype=BF)
        if pn != "N":
            wf[64:69] = wf_h[:5]
        if pn != "P":
            wf[69:74] = wf_h[5:]
        return wf

    return {
        "wf": band(k, R_IN, S_F),
        "w31": band(k, R_OUT, S_B),
        "w11n": band(k, R_IN, -S_B),
        "wbh": mk_bh(""), "wbhP": mk_bh("P"), "wbhN": mk_bh("N"),
        "wfh": mk_fh(""), "wfhP": mk_fh("P"), "wfhN": mk_fh("N"),
    }


@with_exitstack
def _cfar_tile_kernel(ctx, tc, x_d, o_d, w_d, n_img):
    nc = tc.nc
    total = n_img * CHUNKS
    LOOK = 3  # produce lookahead in chunks
    GBUFS = 4

    const = ctx.enter_context(tc.tile_pool(name="const", bufs=1))
    wt = {}
    for name, dram_ap in w_d.items():
        t = const.tile(list(dram_ap.shape), BF16, tag=name)
        nc.sync.dma_start(t[:], dram_ap)
        wt[name] = t

    xp = ctx.enter_context(tc.tile_pool(name="xp", bufs=3))
    h31p = ctx.enter_context(tc.tile_pool(name="h31p", bufs=7))
    h11p = ctx.enter_context(tc.tile_pool(name="h11p", bufs=7))
    gp = ctx.enter_context(tc.tile_pool(name="gp", bufs=GBUFS))
    pp = ctx.enter_context(tc.tile_pool(name="pp", bufs=2, space="PSUM"))
    rp = ctx.enter_context(tc.tile_pool(name="rp", bufs=3))
    obp = ctx.enter_context(tc.tile_pool(name="obp", bufs=2))

    # zero the gather buffers once; after this only halo rows are ever
    # rewritten, so the padding rows stay exactly 0 for the whole run
    gring = []
    for i in range(GBUFS):
        g = gp.tile([128, W], BF16, tag="g")
        nc.gpsimd.memset(g[:], 0.0)
        gring.append(g)

    xts: dict[int, object] = {}
    h31s: dict[int, object] = {}
    h11s: dict[int, object] = {}
    gs: dict[int, object] = {}
    ob0: dict[int, object] = {}
    ob1: dict[int, object] = {}

    def produce(c):
        if c % 2 == 0:
            xt = xp.tile([128, XW], BF16, tag="xt")
            img, tb = c // CHUNKS, (c % CHUNKS) // 2
            src = x_d[img, 256 * tb : 256 * (tb + 1), :].rearrange(
                "(c p) w -> p c w", c=2
            )
            nc.sync.dma_start(xt[:].rearrange("p (c s) -> p c s", c=2), src)
            xts[c] = xts[c + 1] = xt
        xo = (c % 2) * SCANW
        xt = xts[c]
        h31 = h31p.tile([128, N31], BF16, tag="h31")
        nc.vector._custom_dve(
            BOXDIFF, out=h31[:], in0=xt[:, xo + 31 : xo + 31 + N31], in1=xt[:, xo : xo + N31]
        )
        h11 = h11p.tile([128, N11], BF16, tag="h11")
        nc.vector._custom_dve(
            BOXDIFF, out=h11[:], in0=xt[:, xo + 11 : xo + 11 + N11], in1=xt[:, xo : xo + N11]
        )
        h31s[c] = h31
        h11s[c] = h11

    def gather(c):
        lc = c % CHUNKS
        g = gring[c % GBUFS]
        if lc > 0:
            nc.gpsimd.dma_start(g[G31P, :], h31s[c - 1][113:128, O31 : O31 + W])
            nc.gpsimd.dma_start(g[G11P, :], h11s[c - 1][123:128, O11 : O11 + W])
        if lc < CHUNKS - 1:
            nc.gpsimd.dma_start(g[G31N, :], h31s[c + 1][0:15, O31 : O31 + W])
            nc.gpsimd.dma_start(g[G11N, :], h11s[c + 1][0:5, O11 : O11 + W])
        gs[c] = g

    def consume(c):
        img, lc = c // CHUNKS, c % CHUNKS
        sfx = "N" if lc == 0 else ("P" if lc == CHUNKS - 1 else "")
        wbh, wfh = wt["wbh" + sfx], wt["wfh" + sfx]
        g = gs.pop(c)

        psf = pp.tile([128, W], F32, tag="front")
        psb = pp.tile([128, W], F32, tag="back")
        MM = nc.tensor.matmul
        h31, h11 = h31s[c], h11s[c]
        SL = (slice(0, 512), slice(512, 1024))
        # weight-major order: one LDWEIGHTS per weight matrix
        for s in SL:
            MM(psf[:, s], wt["wf"][:], h11[:, O11 + s.start : O11 + s.stop],
               start=True, stop=False)
        for s in SL:
            MM(psf[:, s], wfh[64:128, :], g[64:128, s], start=False, stop=True)
        for s in SL:
            MM(psb[:, s], wt["w31"][:], h31[:, O31 + s.start : O31 + s.stop],
               start=True, stop=False)
        for s in SL:
            MM(psb[:, s], wt["w11n"][:], h11[:, O11 + s.start : O11 + s.stop],
               start=False, stop=False)
        for s in SL:
            MM(psb[:, s], wbh[:], g[:, s], start=False, stop=True)

        p = lc // 2
        if lc % 2 == 0:
            ob0[p] = obp.tile([128, 2 * W], BF16, tag="ob0", name=f"ob0_{img}_{p}")
            ob1[p] = obp.tile([128, 2 * W], BF16, tag="ob1", name=f"ob1_{img}_{p}")
        col = lc % 2
        o0 = ob0[p][:, col * W : (col + 1) * W]
        o1 = ob1[p][:, col * W : (col + 1) * W]
        r = rp.tile([128, W], F32, tag="r")
        nc.vector.reciprocal_approx_fast(out=r[:], in_=psb[:])
        nc.scalar.mul(o1, psf[:], C_FRONT)
        nc.gpsimd.tensor_mul(o0, o1, r[:])
        if col == 1:
            d0 = o_d[img, 256 * p : 256 * (p + 1), :].rearrange(
                "(c q) w -> q c w", c=2
            )
            d1 = o_d[n_img + img, 256 * p : 256 * (p + 1), :].rearrange(
                "(c q) w -> q c w", c=2
            )
            nc.scalar.dma_start(d0, ob0[p][:].rearrange("q (c w) -> q c w", c=2))
            nc.scalar.dma_start(d1, ob1[p][:].rearrange("q (c w) -> q c w", c=2))

    for i in range(total + LOOK):
        if i < total:
            produce(i)
        if 1 <= i <= total:
            gather(i - 1)
        if i >= LOOK:
            consume(i - LOOK)


def build(n_img: int = BPC):
    nc = bacc.Bacc("TRN2", target_bir_lowering=False, debug=False)
    x_d = nc.dram_tensor("x", [n_img, H, SCANW], BF16, kind="ExternalInput").ap()
    o_d = nc.dram_tensor("out", [2 * n_img, H, W], BF16, kind="ExternalOutput").ap()
    wts = _weights()
    w_d = {
        k: nc.dram_tensor(k, list(v.shape), BF16, kind="ExternalInput").ap()
        for k, v in wts.items()
    }
    with tile.TileContext(nc) as tc:
        _cfar_tile_kernel(tc, x_d, o_d, w_d, n_img)
    nc.compile()
    return nc, wts


_CACHE: dict = {}


def make_in_maps(x: np.ndarray, wts: dict) -> list[dict]:
    xs = np.zeros((B, H, SCANW), dtype=BF)
    xs[:, :, PADL : PADL + W] = x[:, 0].astype(BF)
    in_maps = []
    for i in range(NCORES):
        m = {"x": np.ascontiguousarray(xs[BPC * i : BPC * (i + 1)])}
        m.update(wts)
        in_maps.append(m)
    return in_maps


def kernel(x: np.ndarray) -> np.ndarray:
    x = np.ascontiguousarray(np.asarray(x, dtype=np.float32))
    assert x.shape == (B, 1, H, W), x.shape
    if "nc" not in _CACHE:
        _CACHE["nc"], _CACHE["wts"] = build(BPC)
    nc, wts = _CACHE["nc"], _CACHE["wts"]
    in_maps = make_in_maps(x, wts)
    res = run_bass_kernel_spmd(nc, in_maps, list(range(NCORES))).results
    out = np.empty((2 * B, 1, H, W), dtype=np.float32)
    for i in range(NCORES):
        o = np.asarray(res[i]["out"]).astype(np.float32)
        out[BPC * i : BPC * (i + 1), 0] = o[:BPC]
        out[B + BPC * i : B + BPC * (i + 1), 0] = o[BPC:]
    return out


# revision 11
# speedup vs baseline: 1.0874x; 1.0874x over previous
"""CFAR box-filter kernel (31x31 / 11x11 box sums + ratio) for Trainium2.

Data-parallel over batch: 32 images -> 8 NeuronCores, 4 images each.
v3 design:
  - horizontal windowed sums via ONE custom DVE op per window size
    (BOXDIFF: out = cumsum(in0) - cumsum(in1), in0/in1 the same
    zero-padded row shifted by the window length),
  - bf16 datapath end-to-end: bf16 HBM input, bf16 h tiles, bf16
    outputs (host upcasts to fp32),
  - vertical box sums as banded bf16 matmuls; halo terms go through a
    single zero-padded [128,W] gather tile so every matmul keeps a
    128-row moving operand (full PE clock; small moving operands
    drop the PE out of its high p-state),
  - weight scales: wf = bf16(1/121) compensated exactly by the ACT
    copy scale; wb = +-bf16(1/840) leaves a 2.4e-4 constant factor on
    the ratio (far under tolerance),
  - flat chunk loop across images, produce lookahead 3, gathers issued
    from the GpSimd DGE one iteration ahead of their consumer.
"""

import os
import sys

import numpy as np

for _p in ("/opt/trn_rl_repo", "/root/.axon_site/_ro/trn_rl_repo"):
    if os.path.isdir(_p) and _p not in sys.path:
        sys.path.insert(0, _p)
        break

import ml_dtypes

import concourse.bass as bass
import concourse.tile as tile
from concourse import bacc, dve_ops, mybir
from concourse._compat import with_exitstack
from concourse.bass_utils import run_bass_kernel_spmd
from concourse.dve_spec import AluOp, Spec, Src0, Src1, lower, scan

B, H, W = 32, 1024, 1024
NCORES = 8
BPC = B // NCORES            # images per core
CHUNKS = H // 128            # row chunks per image
TOT = BPC * CHUNKS           # chunks per core
PADL, PADR = 31, 15
SCANW = PADL + W + PADR      # 1070
XW = 2 * SCANW               # two chunks per x-load DMA
N31 = 15 + W                 # h31 tile width (first 15 cols are scan warmup)
N11 = 25 + W                 # h11 tile width (first 25 cols are scan warmup)
O31, O11 = 15, 25            # valid-column offsets into h31/h11

F32 = mybir.dt.float32
BF16 = mybir.dt.bfloat16
BF = ml_dtypes.bfloat16

R_IN, R_OUT = 5, 15
S_F = float(BF(1.0 / 121.0))          # front weight scale (bf16 value)
S_B = float(BF(1.0 / 840.0))          # back weight scale (bf16 value)
C_FRONT = 1.0 / (121.0 * S_F)         # exact fp32 fix-up on the ACT copy
# ratio = psf*recip(psb) = front/back * 1/(840*S_B); 840*S_B = 0.99976...


def _register_boxdiff():
    for op in dve_ops.OPS:
        if op.name == "BOXDIFF_ANT":
            return op
    opcode = dve_ops._CUSTOM_DVE_ROW_BASE + len(dve_ops.OPS)
    spec = Spec(
        body=scan(AluOp.ADD, Src0) - scan(AluOp.ADD, Src1),
        reference=lambda in0, in1, s0, s1, imm2: (
            np.cumsum(in0, -1) - np.cumsum(in1, -1)
        ),
    )
    from concourse.dve_uop import DveOpSpec

    shas = {}
    for ver in ("v3", "v4"):
        s = DveOpSpec(
            name="BOXDIFF_ANT", opcode=opcode, uops=lower(spec, ver=ver), rd1_en=True
        )
        shas[ver] = s.sha(ver)
    op = dve_ops.DveOp("BOXDIFF_ANT", spec, subdim=False, uops_sha=shas)
    dve_ops.OPS.append(op)
    dve_ops.CUSTOM_DVE_SPECS[op.name] = spec
    dve_ops._SUB_OPCODE_FOR_NAME[op.name] = opcode
    return op


BOXDIFF = _register_boxdiff()

# gather-tile row layout (fixed): h31 halo at [0:30], h11 halo at [64:74]
G31P, G31N = slice(0, 15), slice(15, 30)
G11P, G11N = slice(64, 69), slice(69, 74)


def _weights() -> dict[str, np.ndarray]:
    k = np.arange(128)[:, None]
    m = np.arange(128)[None, :]
    g11 = np.arange(2 * R_IN)[:, None]
    g11 = np.where(g11 < R_IN, g11 - R_IN, 128 + (g11 - R_IN))
    g31 = np.arange(2 * R_OUT)[:, None]
    g31 = np.where(g31 < R_OUT, g31 - R_OUT, 128 + (g31 - R_OUT))

    def band(gg, radius, scale):
        return ((np.abs(gg - m) <= radius) * scale).astype(BF)

    wb31_h = band(g31, R_OUT, S_B)       # [30,128] rows: prev15, next15
    wn11_h = band(g11, R_IN, -S_B)       # [10,128] rows: prev5, next5
    wf_h = band(g11, R_IN, S_F)          # [10,128]

    def mk_bh(pn):
        wb = np.zeros((128, 128), dtype=BF)
        if pn != "N":
            wb[0:15] = wb31_h[:15]
            wb[G11P] = wn11_h[:5]
        if pn != "P":
            wb[15:30] = wb31_h[15:]
            wb[G11N] = wn11_h[5:]
        return wb

    def mk_fh(pn):
        # stored as [128,128]; the matmul uses rows [64:128] so the weight
        # tile's base partition matches the gather tile's h11-halo rows
        wf = np.zeros((128, 128), dtype=BF)
        if pn != "N":
            wf[64:69] = wf_h[:5]
        if pn != "P":
            wf[69:74] = wf_h[5:]
        return wf

    return {
        "wf": band(k, R_IN, S_F),
        "w31": band(k, R_OUT, S_B),
        "w11n": band(k, R_IN, -S_B),
        "wbh": mk_bh(""), "wbhP": mk_bh("P"), "wbhN": mk_bh("N"),
        "wfh": mk_fh(""), "wfhP": mk_fh("P"), "wfhN": mk_fh("N"),
    }


@with_exitstack
def _cfar_tile_kernel(ctx, tc, x_d, o_d, w_d, n_img):
    nc = tc.nc
    total = n_img * CHUNKS
    LOOK = 3  # produce lookahead in chunks
    GBUFS = 4

    const = ctx.enter_context(tc.tile_pool(name="const", bufs=1))
    wt = {}
    for name, dram_ap in w_d.items():
        t = const.tile(list(dram_ap.shape), BF16, tag=name)
        nc.sync.dma_start(t[:], dram_ap)
        wt[name] = t

    xp = ctx.enter_context(tc.tile_pool(name="xp", bufs=3))
    h31p = ctx.enter_context(tc.tile_pool(name="h31p", bufs=7))
    h11p = ctx.enter_context(tc.tile_pool(name="h11p", bufs=7))
    gp = ctx.enter_context(tc.tile_pool(name="gp", bufs=GBUFS))
    pp = ctx.enter_context(tc.tile_pool(name="pp", bufs=2, space="PSUM"))
    rp = ctx.enter_context(tc.tile_pool(name="rp", bufs=3))
    obp = ctx.enter_context(tc.tile_pool(name="obp", bufs=2))

    # zero the gather buffers once; after this only halo rows are ever
    # rewritten, so the padding rows stay exactly 0 for the whole run
    gring = []
    for i in range(GBUFS):
        g = gp.tile([128, W], BF16, tag="g")
        nc.gpsimd.memset(g[:], 0.0)
        gring.append(g)

    xts: dict[int, object] = {}
    h31s: dict[int, object] = {}
    h11s: dict[int, object] = {}
    gs: dict[int, object] = {}
    ob0: dict[int, object] = {}
    ob1: dict[int, object] = {}

    def produce(c):
        if c % 2 == 0:
            xt = xp.tile([128, XW], BF16, tag="xt")
            img, tb = c // CHUNKS, (c % CHUNKS) // 2
            src = x_d[img, 256 * tb : 256 * (tb + 1), :].rearrange(
                "(c p) w -> p c w", c=2
            )
            nc.sync.dma_start(xt[:].rearrange("p (c s) -> p c s", c=2), src)
            xts[c] = xts[c + 1] = xt
        xo = (c % 2) * SCANW
        xt = xts[c]
        h31 = h31p.tile([128, N31], BF16, tag="h31")
        nc.vector._custom_dve(
            BOXDIFF, out=h31[:], in0=xt[:, xo + 31 : xo + 31 + N31], in1=xt[:, xo : xo + N31]
        )
        h11 = h11p.tile([128, N11], BF16, tag="h11")
        nc.vector._custom_dve(
            BOXDIFF, out=h11[:], in0=xt[:, xo + 11 : xo + 11 + N11], in1=xt[:, xo : xo + N11]
        )
        h31s[c] = h31
        h11s[c] = h11

    def gather(c):
        lc = c % CHUNKS
        g = gring[c % GBUFS]
        if lc > 0:
            nc.sync.dma_start(g[G31P, :], h31s[c - 1][113:128, O31 : O31 + W])
            nc.sync.dma_start(g[G11P, :], h11s[c - 1][123:128, O11 : O11 + W])
        if lc < CHUNKS - 1:
            nc.scalar.dma_start(g[G31N, :], h31s[c + 1][0:15, O31 : O31 + W])
            nc.scalar.dma_start(g[G11N, :], h11s[c + 1][0:5, O11 : O11 + W])
        gs[c] = g

    def consume(c):
        img, lc = c // CHUNKS, c % CHUNKS
        sfx = "N" if lc == 0 else ("P" if lc == CHUNKS - 1 else "")
        wbh, wfh = wt["wbh" + sfx], wt["wfh" + sfx]
        g = gs.pop(c)

        psf = pp.tile([128, W], F32, tag="front")
        psb = pp.tile([128, W], F32, tag="back")
        MM = nc.tensor.matmul
        h31, h11 = h31s[c], h11s[c]
        SL = (slice(0, 512), slice(512, 1024))
        # weight-major order: one LDWEIGHTS per weight matrix
        for s in SL:
            MM(psf[:, s], wt["wf"][:], h11[:, O11 + s.start : O11 + s.stop],
               start=True, stop=False)
        for s in SL:
            MM(psf[:, s], wfh[64:128, :], g[64:128, s], start=False, stop=True)
        for s in SL:
            MM(psb[:, s], wt["w31"][:], h31[:, O31 + s.start : O31 + s.stop],
               start=True, stop=False)
        for s in SL:
            MM(psb[:, s], wt["w11n"][:], h11[:, O11 + s.start : O11 + s.stop],
               start=False, stop=False)
        for s in SL:
            MM(psb[:, s], wbh[:], g[:, s], start=False, stop=True)

        p = lc // 2
        if lc % 2 == 0:
            ob0[p] = obp.tile([128, 2 * W], BF16, tag="ob0", name=f"ob0_{img}_{p}")
            ob1[p] = obp.tile([128, 2 * W], BF16, tag="ob1", name=f"ob1_{img}_{p}")
        col = lc % 2
        o0 = ob0[p][:, col * W : (col + 1) * W]
        o1 = ob1[p][:, col * W : (col + 1) * W]
        r = rp.tile([128, W], BF16, tag="r")
        # direct custom-dve call: fp32 PSUM input (the BITWISE_NOT seed needs
        # fp32 bits), bf16 output cast on write
        rc = dve_ops.RECIP_APPROX_FAST_CONSTS
        nc.vector._custom_dve(
            dve_ops.RECIPROCAL_APPROX_FAST, out=r[:], in0=psb[:],
            s0=rc["s0"], s1=rc["s1"], imm2=rc["imm2"],
        )
        nc.scalar.mul(o1, psf[:], C_FRONT)
        nc.gpsimd.tensor_mul(o0, o1, r[:])
        if col == 1:
            d0 = o_d[img, 256 * p : 256 * (p + 1), :].rearrange(
                "(c q) w -> q c w", c=2
            )
            d1 = o_d[n_img + img, 256 * p : 256 * (p + 1), :].rearrange(
                "(c q) w -> q c w", c=2
            )
            nc.scalar.dma_start(d0, ob0[p][:].rearrange("q (c w) -> q c w", c=2))
            nc.scalar.dma_start(d1, ob1[p][:].rearrange("q (c w) -> q c w", c=2))

    for i in range(total + LOOK):
        if i < total:
            produce(i)
        if 1 <= i <= total:
            gather(i - 1)
        if i >= LOOK:
            consume(i - LOOK)


def build(n_img: int = BPC):
    nc = bacc.Bacc("TRN2", target_bir_lowering=False, debug=False)
    x_d = nc.dram_tensor("x", [n_img, H, SCANW], BF16, kind="ExternalInput").ap()
    o_d = nc.dram_tensor("out", [2 * n_img, H, W], BF16, kind="ExternalOutput").ap()
    wts = _weights()
    w_d = {
        k: nc.dram_tensor(k, list(v.shape), BF16, kind="ExternalInput").ap()
        for k, v in wts.items()
    }
    with tile.TileContext(nc) as tc:
        _cfar_tile_kernel(tc, x_d, o_d, w_d, n_img)
    nc.compile()
    return nc, wts


_CACHE: dict = {}


def make_in_maps(x: np.ndarray, wts: dict) -> list[dict]:
    xs = np.zeros((B, H, SCANW), dtype=BF)
    xs[:, :, PADL : PADL + W] = x[:, 0].astype(BF)
    in_maps = []
    for i in range(NCORES):
        m = {"x": np.ascontiguousarray(xs[BPC * i : BPC * (i + 1)])}
        m.update(wts)
        in_maps.append(m)
    return in_maps


def kernel(x: np.ndarray) -> np.ndarray:
    x = np.ascontiguousarray(np.asarray(x, dtype=np.float32))
    assert x.shape == (B, 1, H, W), x.shape
    if "nc" not in _CACHE:
        _CACHE["nc"], _CACHE["wts"] = build(BPC)
    nc, wts = _CACHE["nc"], _CACHE["wts"]
    in_maps = make_in_maps(x, wts)
    res = run_bass_kernel_spmd(nc, in_maps, list(range(NCORES))).results
    out = np.empty((2 * B, 1, H, W), dtype=np.float32)
    for i in range(NCORES):
        o = np.asarray(res[i]["out"]).astype(np.float32)
        out[BPC * i : BPC * (i + 1), 0] = o[:BPC]
        out[B + BPC * i : B + BPC * (i + 1), 0] = o[BPC:]
    return out
